# revision 1
# baseline (speedup 1.0000x reference)
"""Causal self-attention (B=2, L=2048, D=1024, H=16) on 8 Trainium2 NeuronCores.

Sharding: batch x head-group data/tensor parallel. Core c handles batch
c//4 and heads 4*(c%4)..4*(c%4)+3. w_qkv is column-sharded by head,
w_proj row-sharded; the output all-reduce (sum of per-core partials
within each batch group) is done on the host after the gather.

Per-core pipeline (all matmuls on TensorE, fp32r except P@V in bf16):
  phase 1: qT/kT = (x @ Wq|Wk)^T directly in transposed layout
  phase 2: v = x @ Wv in natural layout (+ ones column for softmax sums)
  phase 3: per (q-supertile, head): S^T tiles = k_chunk @ q^T, exp on
           ScalarE (scale=1/8), causal zeroing via gpsimd affine_select,
           P@V with an appended ones row accumulating [o'; sums] in PSUM,
           normalize with approx-reciprocal + partition broadcast
  phase 4: out_partial = o_heads @ w_proj_local, streamed to DRAM
"""
import os
import sys
from contextlib import ExitStack

for _p in ("/opt/trn_rl_repo", "/root/.axon_site/_ro/trn_rl_repo"):
    if os.path.isdir(_p) and _p not in sys.path:
        sys.path.append(_p)

import numpy as np

import concourse.bass as bass  # noqa: F401  (registers AP types)
import concourse.mybir as mybir
import concourse.tile as tile
from concourse import bacc
from concourse.bass_utils import run_bass_kernel_spmd

dt = mybir.dt
F32, F32R, BF16 = dt.float32, dt.float32r, dt.bfloat16
EXP = mybir.ActivationFunctionType.Exp
GE = mybir.AluOpType.is_ge

L = 2048          # sequence length
D = 1024          # model dim
DH = 64           # head dim
HL = 4            # local heads per core
DC = D // 128     # 8 contraction chunks of 128
NQC = L // 512    # 4 q supertiles
NQB = L // 128    # 16 q blocks
N_CORES = 8

_nc_cache = []


def _emit(nc):
    xT = nc.dram_tensor("xT", [D, L], F32R, kind="ExternalInput").ap()
    wqk = nc.dram_tensor("wqk", [D, 512], F32R, kind="ExternalInput").ap()
    wv = nc.dram_tensor("wv", [D, 256], F32R, kind="ExternalInput").ap()
    wp = nc.dram_tensor("wp", [128, 2 * D], F32R, kind="ExternalInput").ap()
    out = nc.dram_tensor("out", [L, D], F32, kind="ExternalOutput").ap()

    with tile.TileContext(nc) as tc, ExitStack() as ctx:
        const = ctx.enter_context(tc.tile_pool(name="const", bufs=1))
        xT_sb = const.tile([128, DC * L], F32R)
        wqk_sb = const.tile([128, DC * 512], F32R)
        wv_sb = const.tile([128, DC * 256], F32R)
        wp_sb = const.tile([128, 2 * D], F32R)
        qkT_sb = const.tile([128, 4 * L], F32R)   # [qT01|qT23|kT01|kT23]
        v_sb = const.tile([128, NQB * HL, DH + 1], BF16)
        oT01 = const.tile([128, L], F32R)
        oT23 = const.tile([128, L], F32R)

        # input loads, interleaved so phase 1 can start on chunk 0 early
        for c in range(DC):
            nc.sync.dma_start(out=wqk_sb[:, c * 512:(c + 1) * 512],
                              in_=wqk[c * 128:(c + 1) * 128, :])
            nc.sync.dma_start(out=wv_sb[:, c * 256:(c + 1) * 256],
                              in_=wv[c * 128:(c + 1) * 128, :])
            nc.sync.dma_start(out=xT_sb[:, c * L:(c + 1) * L],
                              in_=xT[c * 128:(c + 1) * 128, :])
        nc.sync.dma_start(out=wp_sb[:], in_=wp[:])
        nc.gpsimd.memset(v_sb[:, :, DH:DH + 1], 1.0)

        # phase 1: qkT blocks (cb: 0=qT h01, 1=qT h23, 2=kT h01, 3=kT h23)
        with tc.tile_pool(name="ps_qk", bufs=3, space="PSUM") as ps_qk:
            for cb in range(4):
                for qt in range(NQC):
                    pt = ps_qk.tile([128, 512], F32)
                    for c in range(DC):
                        nc.tensor.matmul(
                            pt[:],
                            wqk_sb[:, c * 512 + cb * 128: c * 512 + (cb + 1) * 128],
                            xT_sb[:, c * L + qt * 512: c * L + qt * 512 + 512],
                            start=(c == 0), stop=(c == DC - 1))
                    nc.vector.tensor_copy(
                        qkT_sb[:, cb * L + qt * 512: cb * L + qt * 512 + 512],
                        pt[:])

        # phase 2: v natural layout, bf16, with per-(chunk,head) ones column
        with tc.tile_pool(name="ps_v", bufs=3, space="PSUM") as ps_v:
            for qb in range(NQB):
                pt = ps_v.tile([128, 256], F32)
                for c in range(DC):
                    nc.tensor.matmul(
                        pt[:],
                        xT_sb[:, c * L + qb * 128: c * L + (qb + 1) * 128],
                        wv_sb[:, c * 256:(c + 1) * 256],
                        start=(c == 0), stop=(c == DC - 1))
                nc.vector.tensor_copy(
                    v_sb[:, qb * HL:(qb + 1) * HL, 0:DH],
                    pt[:].rearrange("p (h d) -> p h d", d=DH))

        # phases 3+4 per q supertile
        ps_s = ctx.enter_context(tc.tile_pool(name="ps_s", bufs=2, space="PSUM"))
        ps_o = ctx.enter_context(tc.tile_pool(name="ps_o", bufs=2, space="PSUM"))
        ps_f = ctx.enter_context(tc.tile_pool(name="ps_f", bufs=2, space="PSUM"))
        expp = ctx.enter_context(tc.tile_pool(name="expp", bufs=3))
        rp = ctx.enter_context(tc.tile_pool(name="rp", bufs=2))
        outp = ctx.enter_context(tc.tile_pool(name="outp", bufs=3))

        for qc in range(NQC):
            nkc = 4 * qc + 4
            for h in range(HL):
                pb = 64 * (h % 2)
                qT_off = (h // 2) * L + qc * 512
                kT_off = (2 + h // 2) * L
                po = ps_o.tile([128, 512], F32)

                def s_group(g):
                    """Emit the 2 S^T matmuls + exp + causal select for
                    k-chunk pair g; returns the bf16 expS tile."""
                    st = ps_s.tile([128, 1024], F32, tag="st")
                    et = expp.tile([128, 1024], BF16, tag="et")
                    for j in range(2):
                        kc = 2 * g + j
                        nc.tensor.matmul(
                            st[:, j * 512:(j + 1) * 512],
                            qkT_sb[pb:pb + 64, kT_off + kc * 128: kT_off + (kc + 1) * 128],
                            qkT_sb[pb:pb + 64, qT_off: qT_off + 512],
                            start=True, stop=True)
                    nc.scalar.activation(et[:], st[:], EXP, scale=0.125)
                    for j in range(2):
                        kc = 2 * g + j
                        if kc >= 4 * qc:  # diagonal chunk: zero q < k
                            m = 128 * kc - 512 * qc
                            nc.gpsimd.affine_select(
                                out=et[:, j * 512:(j + 1) * 512],
                                in_=et[:, j * 512:(j + 1) * 512],
                                compare_op=GE, fill=0.0, base=-m,
                                pattern=[[1, 512]], channel_multiplier=-1)
                    return et

                def pv(g, et):
                    for j in range(2):
                        kc = 2 * g + j
                        nc.tensor.matmul(
                            po[0:DH + 1, :],
                            v_sb[:, kc * HL + h, :],
                            et[:, j * 512:(j + 1) * 512],
                            start=(kc == 0), stop=(kc == nkc - 1))

                # software-pipelined: S(g+1) overlaps exp/select/PV of g
                prev = None
                for g in range(nkc // 2):
                    et = s_group(g)
                    if prev is not None:
                        pv(g - 1, prev)
                    prev = et
                pv(nkc // 2 - 1, prev)

                # normalize: oT[:, qc] = o' * (1/sums) broadcast over dh
                rs = rp.tile([1, 512], F32, tag="rs")
                r1 = rp.tile([1, 512], F32, tag="r1")
                r64 = rp.tile([64, 512], F32, tag="r64")
                nc.vector.tensor_copy(rs[:], po[DH:DH + 1, :])
                nc.vector.reciprocal_approx_fast(r1[:], rs[:])
                nc.gpsimd.partition_broadcast(r64[:], r1[:])
                oT = oT01 if h < 2 else oT23
                nc.vector.tensor_mul(oT[pb:pb + 64, qc * 512:(qc + 1) * 512],
                                     po[0:DH, :], r64[:])

            # phase 4 for this supertile's 4 q blocks
            for qb in range(4 * qc, 4 * qc + 4):
                for nh in range(2):
                    pf = ps_f.tile([128, 512], F32)
                    nc.tensor.matmul(pf[:], oT01[:, qb * 128:(qb + 1) * 128],
                                     wp_sb[:, nh * 512: nh * 512 + 512],
                                     start=True, stop=False)
                    nc.tensor.matmul(pf[:], oT23[:, qb * 128:(qb + 1) * 128],
                                     wp_sb[:, D + nh * 512: D + nh * 512 + 512],
                                     start=False, stop=True)
                    ot = outp.tile([128, 512], F32)
                    nc.vector.tensor_copy(ot[:], pf[:])
                    nc.sync.dma_start(
                        out=out[qb * 128:(qb + 1) * 128, nh * 512:(nh + 1) * 512],
                        in_=ot[:])


def _get_nc():
    if not _nc_cache:
        nc = bacc.Bacc("TRN2", debug=False, target_bir_lowering=False)
        _emit(nc)
        nc.compile()
        _nc_cache.append(nc)
    return _nc_cache[0]


def make_in_maps(x, w_qkv, w_proj):
    """Host-side sharding: per-core input dict (all contiguous fp32)."""
    x = np.asarray(x, dtype=np.float32)
    w_qkv = np.asarray(w_qkv, dtype=np.float32)
    w_proj = np.asarray(w_proj, dtype=np.float32)
    in_maps = []
    for c in range(N_CORES):
        b = c // 4
        hb = 256 * (c % 4)  # column offset of this core's 4 heads
        xTc = np.ascontiguousarray(x[b].T)
        wqk_c = np.ascontiguousarray(np.concatenate(
            [w_qkv[:, hb:hb + 256], w_qkv[:, D + hb: D + hb + 256]], axis=1))
        wv_c = np.ascontiguousarray(w_qkv[:, 2 * D + hb: 2 * D + hb + 256])
        wpl = w_proj[hb:hb + 256, :]
        wp_c = np.ascontiguousarray(np.concatenate([wpl[0:128], wpl[128:256]],
                                                   axis=1))
        in_maps.append({"xT": xTc, "wqk": wqk_c, "wv": wv_c, "wp": wp_c})
    return in_maps


def combine_outputs(outs):
    """Sum per-core partials within each batch group (host all-reduce)."""
    o0 = outs[0] + outs[1] + outs[2] + outs[3]
    o1 = outs[4] + outs[5] + outs[6] + outs[7]
    return np.stack([o0, o1]).astype(np.float32)


def kernel(x, w_qkv, w_proj):
    nc = _get_nc()
    in_maps = make_in_maps(x, w_qkv, w_proj)
    res = run_bass_kernel_spmd(nc, in_maps, list(range(N_CORES)))
    return combine_outputs([r["out"] for r in res.results])
